# revision 53
# baseline (speedup 1.0000x reference)
"""AttentionBlock3D (GroupNorm + 8-head attention + proj + residual) on 8 trn2 cores.

Sharding: core i handles (batch b = i//4, query-quarter qs = i%4).
Each core redundantly computes GroupNorm + full K/V for its batch (cheap:
~13% extra FLOPs), and exclusively computes Q/attention/projection for its
1024 spatial positions. No inter-core communication; the host concatenates
the 8 output slices.

Per-core design (ACT-bound: 33.5M softmax exps at 1 elem/lane/cycle):
  - All matmuls in fp32r (full PE rate at moving-dim >= 256, ~19-bit acc).
  - hn [C,N], K [C,N], Q [C,NQ] channel-major: head h's k^T/q^T are rows
    32h..32h+32, directly usable as QK^T operands -- no transposes anywhere.
  - QK^T pairs two heads via tile_position row groups (32-wide K strips).
  - V^T [N, 8, 33]: 32 value columns + a ones column per head, so one AV
    matmul accumulates both O^T and the softmax denominator (row 32).
  - Softmax skips the max-subtraction (scores are O(1) by construction).
  - exp(scale*s) on ACT reads scores straight from PSUM [128,1024] and
    writes fp32r SBUF tiles that feed AV.
  - AV matmuls lag the QK/exp stream by one step so wave boundaries never
    stall the exp pipeline on the in-order PE queue.
  - Denominator reciprocal is broadcast across dh rows by the idle GPSIMD
    engine (physical-partition-0 broadcast).
  - GroupNorm: bn_stats + per-group combine via a tiny one-hot matmul;
    gamma/beta and 1/sqrt(dh) are folded into weights/exp-scale on the host.
"""

import numpy as np

B, C, N = 2, 256, 4096
HEADS, GROUPS = 8, 8
DH = C // HEADS  # 32
NQ = N // 4      # queries per core
EPS = 1e-5
N_CORES = 8
NKB = N // 128   # 32 key blocks
SCALE = 1.0 / float(np.sqrt(DH))

LAST_RESULTS = None  # BassKernelResults of the most recent run (for test.py)


def _build_program():
    import concourse.bass as bass
    import concourse.bacc as bacc
    import concourse.tile as tile
    from concourse import mybir

    f32 = mybir.dt.float32
    f32r = mybir.dt.float32r
    Alu = mybir.AluOpType
    Act = mybir.ActivationFunctionType

    nc = bass.Bass("TRN2", target_bir_lowering=False, use_seq_codegen=True)

    # ---- DRAM I/O ----
    x_d = nc.dram_tensor("x", [C, N], f32, kind="ExternalInput")
    xq_d = nc.dram_tensor("xq", [C, NQ], f32, kind="ExternalInput")
    wqT_d = nc.dram_tensor("wqT", [C, C], f32, kind="ExternalInput")
    wkT_d = nc.dram_tensor("wkT", [C, C], f32, kind="ExternalInput")
    wvT_d = nc.dram_tensor("wvT", [C, C], f32, kind="ExternalInput")
    wpT_d = nc.dram_tensor("wpT", [C, C], f32, kind="ExternalInput")
    bq_d = nc.dram_tensor("bq", [C, 1], f32, kind="ExternalInput")
    bk_d = nc.dram_tensor("bk", [C, 1], f32, kind="ExternalInput")
    bv_d = nc.dram_tensor("bv", [1, C], f32, kind="ExternalInput")
    bp_d = nc.dram_tensor("bp", [C, 1], f32, kind="ExternalInput")
    gmap_d = nc.dram_tensor("gmap", [2, 128, GROUPS], f32, kind="ExternalInput")
    bmap_d = nc.dram_tensor("bmap", [2, GROUPS, 128], f32, kind="ExternalInput")
    out_d = nc.dram_tensor("out", [C, NQ], f32, kind="ExternalOutput")

    with tile.TileContext(nc) as tc:
        with (
            tc.tile_pool(name="const", bufs=1) as const,
            tc.tile_pool(name="data", bufs=1) as data,
            tc.tile_pool(name="tmp", bufs=2) as tmp,
            tc.tile_pool(name="exps", bufs=3) as exps,
            tc.tile_pool(name="psA", bufs=2, space="PSUM") as psA,
            tc.tile_pool(name="psB", bufs=2, space="PSUM") as psB,
            tc.tile_pool(name="psC", bufs=2, space="PSUM") as psC,
        ):
            # ---- small constants (needed by the GN stats path) ----
            bq_sb = [const.tile([128, 1], f32, name=f"bq{j}") for j in range(2)]
            bk_sb = [const.tile([128, 1], f32, name=f"bk{j}") for j in range(2)]
            bp_sb = [const.tile([128, 1], f32, name=f"bp{j}") for j in range(2)]
            for j in range(2):
                nc.gpsimd.dma_start(out=bq_sb[j], in_=bq_d[j * 128:(j + 1) * 128, :])
                nc.gpsimd.dma_start(out=bk_sb[j], in_=bk_d[j * 128:(j + 1) * 128, :])
                nc.gpsimd.dma_start(out=bp_sb[j], in_=bp_d[j * 128:(j + 1) * 128, :])
            bv_sb = const.tile([128, C], f32)
            nc.gpsimd.dma_start(out=bv_sb, in_=bv_d[:, :].to_broadcast([128, C]))
            gmap_sb = [const.tile([128, GROUPS], f32, name=f"gmap{j}") for j in range(2)]
            bmap_sb = [const.tile([GROUPS, 128], f32, name=f"bmap{j}") for j in range(2)]
            for j in range(2):
                gstg = tmp.tile([128, GROUPS], f32, tag="gstg", name="gstg", bufs=2)
                nc.gpsimd.dma_start(out=gstg, in_=gmap_d[j])
                nc.vector.tensor_copy(out=gmap_sb[j], in_=gstg)
                bstg = tmp.tile([GROUPS, 128], f32, tag="bstg", name="bstg", bufs=2)
                nc.gpsimd.dma_start(out=bstg, in_=bmap_d[j])
                nc.vector.tensor_copy(out=bmap_sb[j], in_=bstg)

            # ACT table prewarm: a dummy exp at t=0 pulls the ln/exp table
            # load off the critical path (GN rstd and softmax share the set)
            warm = tmp.tile([8, 1], f32, tag="warm", bufs=1)
            nc.vector.memset(warm, 0.0)
            nc.scalar.activation(out=warm, in_=warm, func=Act.Exp)

            # ---- load x (chunked so stats start before the full load) ----
            xt = [data.tile([128, N], f32, name=f"xt{j}") for j in range(2)]
            xqt = [data.tile([128, NQ], f32, name=f"xqt{j}") for j in range(2)]
            for j in range(2):
                eng = nc.sync if j == 0 else nc.scalar
                for ch in range(4):
                    csl = slice(ch * 1024, (ch + 1) * 1024)
                    eng.dma_start(out=xt[j][:, csl], in_=x_d[j * 128:(j + 1) * 128, csl])
                eng.dma_start(out=xqt[j], in_=xq_d[j * 128:(j + 1) * 128, :])

            # normalized activations (fp32r, produced by the affine pass below)
            hn = [data.tile([128, N], f32r, name=f"hn{j}") for j in range(2)]
            hnq = [data.tile([128, NQ], f32r, name=f"hnq{j}") for j in range(2)]

            # ---- GroupNorm statistics via bn_stats (one DVE pass over x) ----
            # st[j] = per-partition [mean, E[x^2]]
            st = [tmp.tile([128, 2], f32, name=f"st{j}", tag="st", bufs=2) for j in range(2)]
            for j in range(2):
                bnst = tmp.tile([128, 8, 6], f32, tag="bnst", bufs=2, name="bnst")
                for sub in range(8):
                    nc.vector.bn_stats(
                        out=bnst[:, sub, :],
                        in_=xt[j][:, sub * 512:(sub + 1) * 512],
                    )
                mv = tmp.tile([128, 2], f32, tag="mv", bufs=2, name="mv")
                nc.vector.bn_aggr(out=mv, in_=bnst)
                nc.vector.tensor_copy(out=st[j][:, 0:1], in_=mv[:, 0:1])
                nc.vector.tensor_mul(out=st[j][:, 1:2], in0=mv[:, 0:1], in1=mv[:, 0:1])
                nc.vector.tensor_add(out=st[j][:, 1:2], in0=st[j][:, 1:2], in1=mv[:, 1:2])
            stats_ps = psC.tile([GROUPS, 2], f32, tag="work")
            for j in range(2):
                nc.tensor.matmul(
                    stats_ps, gmap_sb[j], st[j], start=(j == 0), stop=(j == 1),
                )
            # per-group mean / rstd (tiny DVE/ACT ops on 8 partitions)
            gs = tmp.tile([GROUPS, 2], f32, tag="gs", bufs=1)
            nc.vector.tensor_copy(out=gs, in_=stats_ps)
            inv_n = 1.0 / (C // GROUPS)  # each group sums 32 per-partition means
            ms = tmp.tile([GROUPS, 2], f32, tag="ms", bufs=1)  # [mu | rstd]
            nc.vector.tensor_scalar_mul(out=ms[:, 0:1], in0=gs[:, 0:1], scalar1=inv_n)
            ve = tmp.tile([GROUPS, 1], f32, tag="ve", bufs=1)
            nc.vector.tensor_scalar_mul(out=ve, in0=gs[:, 1:2], scalar1=inv_n)
            musq = tmp.tile([GROUPS, 1], f32, tag="musq", bufs=1)
            nc.vector.tensor_mul(out=musq, in0=ms[:, 0:1], in1=ms[:, 0:1])
            nc.vector.tensor_sub(out=ve, in0=ve, in1=musq)
            nc.vector.tensor_scalar_add(out=ve, in0=ve, scalar1=EPS)
            # rstd = exp(-0.5*ln(v)): Ln/Exp share one activation table set
            # with the softmax exps, so no extra table load on the critical path
            sd = tmp.tile([GROUPS, 1], f32, tag="sd", bufs=1)
            nc.scalar.activation(out=sd, in_=ve, func=Act.Ln)
            r0 = tmp.tile([GROUPS, 1], f32, tag="r0", bufs=1)
            nc.scalar.activation(out=r0, in_=sd, func=Act.Exp, scale=-0.5)
            # one Newton polish: r = r0 * (1.5 - 0.5 * ve * r0^2)
            t_nw = tmp.tile([GROUPS, 1], f32, tag="t_nw", bufs=1)
            nc.vector.tensor_mul(out=t_nw, in0=r0, in1=r0)
            nc.vector.tensor_mul(out=t_nw, in0=t_nw, in1=ve)
            nc.vector.tensor_scalar(
                out=t_nw, in0=t_nw, scalar1=-0.5, scalar2=1.5,
                op0=Alu.mult, op1=Alu.add,
            )
            nc.vector.tensor_mul(out=ms[:, 1:2], in0=r0, in1=t_nw)

            # ---- weights: f32 staging (SP queue), DVE copies deferred ----
            # (walrus requires fp32r matmul operands to be *produced* as fp32r)
            wq_sb = [const.tile([128, C], f32r, name=f"wq{j}") for j in range(2)]
            wk_sb = [const.tile([128, C], f32r, name=f"wk{j}") for j in range(2)]
            wv_sb = [const.tile([128, C], f32r, name=f"wv{j}") for j in range(2)]
            wp_sb = [const.tile([128, C], f32r, name=f"wp{j}") for j in range(2)]
            wstgs = {}
            for j in range(2):
                for wi, (wd, wt) in enumerate(((wqT_d, wq_sb), (wkT_d, wk_sb),
                                               (wvT_d, wv_sb), (wpT_d, wp_sb))):
                    wstg = tmp.tile([128, C], f32, tag="wstg", name="wstg", bufs=8)
                    nc.sync.dma_start(out=wstg, in_=wd[j * 128:(j + 1) * 128, :])
                    wstgs[(wi, j)] = (wstg, wt)

            def copy_w(wi, j):
                wstg, wt = wstgs[(wi, j)]
                nc.vector.tensor_copy(out=wt[j], in_=wstg)

            # broadcast (mu, rstd) to per-partition columns
            musc = []
            for j in range(2):
                bc_ps = psC.tile([128, 2], f32, tag="work", name=f"bc_ps{j}")
                nc.tensor.matmul(bc_ps, bmap_sb[j], ms, start=True, stop=True)
                m = tmp.tile([128, 3], f32, tag="musc", bufs=2, name=f"musc{j}")
                nc.vector.tensor_copy(out=m[:, 0:2], in_=bc_ps)
                nc.vector.tensor_mul(out=m[:, 2:3], in0=m[:, 0:1], in1=m[:, 1:2])
                nc.vector.tensor_scalar_mul(out=m[:, 2:3], in0=m[:, 2:3], scalar1=-1.0)
                musc.append(m)

            # ---- normalize (gamma/beta pre-folded into weights on host) ----
            # xt/xqt stay raw fp32 (xqt doubles as the residual source)
            for j in range(2):
                copy_w(0, j)  # wq
                copy_w(1, j)  # wk
            # hnq normalize on ACT (idle, runs parallel to the DVE hn chain)
            for j in range(2):
                nc.scalar.activation(
                    out=hnq[j], in_=xqt[j], func=Act.Identity,
                    bias=musc[j][:, 2:3], scale=musc[j][:, 1:2],
                )

            def norm_half(hh):
                hsl = slice(hh * (N // 2), (hh + 1) * (N // 2))
                for j in range(2):
                    nc.vector.tensor_scalar(
                        out=hn[j][:, hsl], in0=xt[j][:, hsl],
                        scalar1=musc[j][:, 0:1], scalar2=musc[j][:, 1:2],
                        op0=Alu.subtract, op1=Alu.mult,
                    )

            K_sb = [data.tile([128, N], f32r, name=f"K{j}") for j in range(2)]
            Q_sb = [data.tile([128, NQ], f32r, name=f"Q{j}") for j in range(2)]

            def emit_q(j, n):
                ps = psC.tile([128, 512], f32, tag="work", name="qps")
                for kk in range(2):
                    nc.tensor.matmul(
                        ps,
                        wq_sb[kk][:, j * 128:(j + 1) * 128],
                        hnq[kk][:, n * 512:(n + 1) * 512],
                        start=(kk == 0), stop=(kk == 1),
                    )
                nc.vector.tensor_scalar_add(
                    out=Q_sb[j][:, n * 512:(n + 1) * 512], in0=ps,
                    scalar1=bq_sb[j],
                )

            def emit_k(j, n):
                ps = psC.tile([128, 512], f32, tag="work", name="kps")
                for kk in range(2):
                    nc.tensor.matmul(
                        ps,
                        wk_sb[kk][:, j * 128:(j + 1) * 128],
                        hn[kk][:, n * 512:(n + 1) * 512],
                        start=(kk == 0), stop=(kk == 1),
                    )
                nc.vector.tensor_scalar_add(
                    out=K_sb[j][:, n * 512:(n + 1) * 512], in0=ps,
                    scalar1=bk_sb[j],
                )

            for n in range(NQ // 512):
                emit_q(0, n)
            norm_half(0)
            for n in range(4):
                emit_k(0, n)
            norm_half(1)
            for n in range(4, N // 512):
                emit_k(0, n)
            for j in range(2):
                copy_w(2, j)  # wv
                copy_w(3, j)  # wp

            # ---- V^T = (hn)^T @ Wv^T + bv, layout [128, kb, 8, 33] ----
            # per head: 32 value columns + a ones column, so a single AV matmul
            # ([128, 33] lhsT) also produces the softmax denominator in row 32.
            # The per-kb V matmuls are emitted inside the first attention wave
            # so the exp stream starts as early as possible.
            V_sb = data.tile([128, NKB, HEADS, DH + 1], f32r)
            vones = const.tile([128, NKB * HEADS], f32)
            nc.vector.memset(vones, 1.0)
            nc.vector.tensor_copy(
                out=V_sb[:, :, :, DH:DH + 1],
                in_=vones.rearrange("p (kb h o) -> p kb h o", h=HEADS, o=1),
            )

            def emit_v(kb):
                ps = psC.tile([128, C], f32, tag="work", name="vps")
                for kk in range(2):
                    nc.tensor.matmul(
                        ps,
                        hn[kk][:, kb * 128:(kb + 1) * 128],
                        wv_sb[kk],
                        start=(kk == 0), stop=(kk == 1),
                    )
                nc.vector.tensor_add(
                    out=V_sb[:, kb, :, 0:DH],
                    in0=ps.rearrange("p (h x) -> p h x", h=HEADS),
                    in1=bv_sb.rearrange("p (h x) -> p h x", h=HEADS),
                )

            # ---- attention: chunk-outer so proj/output can drain per chunk.
            # AV matmuls lag the QK/exp stream by one step so the next wave's
            # QKs slot in front of the previous wave's last AV on the in-order
            # PE queue (kills the wave-boundary exp stall). The normalize tail
            # rides the lagged last AV.
            O_sb = [data.tile([128, NQ], f32r, name=f"O{j}") for j in range(2)]
            out_sb = [data.tile([128, NQ], f32, name=f"outsb{j}") for j in range(2)]

            def make_step(info, kb, ex):
                def emit():
                    if info["oda"] is None:
                        info["oda"] = psB.tile([DH + 1, 512], f32, tag="otd", name="oda")
                        info["odb"] = psB.tile([DH + 1, 512], f32, tag="otd", name="odb")
                    first, last = (kb == 0), (kb == NKB - 1)
                    nc.tensor.matmul(
                        info["oda"], V_sb[:, kb, info["hA"], :], ex[:, 0, :],
                        start=first, stop=last, skip_group_check=True,
                        tile_position=(0, 0),
                    )
                    nc.tensor.matmul(
                        info["odb"], V_sb[:, kb, info["hB"], :], ex[:, 1, :],
                        start=first, stop=last, skip_group_check=True,
                        tile_position=(0, 0),
                    )
                    if last:
                        # normalize: O^T * (1/denom); both reciprocals land on
                        # partition 0 (free-dim separated) because gpsimd
                        # partition_broadcast replicates physical partition 0
                        oda, odb = info["oda"], info["odb"]
                        rc = tmp.tile([32, 2, 512], f32, tag="rc", name="rc", bufs=1)
                        nc.vector.reciprocal(out=rc[0:1, 0, :], in_=oda[DH:DH + 1, :])
                        nc.vector.reciprocal(out=rc[0:1, 1, :], in_=odb[DH:DH + 1, :])
                        nb = tmp.tile([32, 2, 512], f32, tag="nb", name="nb", bufs=1)
                        nc.gpsimd.partition_broadcast(nb[:, 0, :], rc[0:1, 0, :])
                        nc.gpsimd.partition_broadcast(nb[:, 1, :], rc[0:1, 1, :])
                        nc.vector.tensor_mul(
                            out=O_sb[info["jt"]][info["sA"]:info["sA"] + 32, info["qsl"]],
                            in0=oda[0:DH, :], in1=nb[:, 0, :],
                        )
                        nc.vector.tensor_mul(
                            out=O_sb[info["jt"]][info["sB"]:info["sB"] + 32, info["qsl"]],
                            in0=odb[0:DH, :], in1=nb[:, 1, :],
                        )
                return emit

            wave_i = 0
            pending = None
            for c in range(NQ // 512):
                qsl = slice(c * 512, (c + 1) * 512)
                for p in range(4):
                    hA, hB = 2 * p, 2 * p + 1
                    info = {
                        "hA": hA, "hB": hB, "jt": hA // 4,
                        "sA": 32 * (hA % 4), "sB": 32 * (hB % 4),
                        "qsl": qsl, "oda": None, "odb": None,
                    }
                    sA, sB, jt = info["sA"], info["sB"], info["jt"]
                    for kb in range(NKB):
                        if wave_i == 0:
                            emit_v(kb)
                        elif wave_i == 2:
                            # K/Q j=1 computed just-in-time for this wave's QKs
                            if kb < 2:
                                emit_q(1, kb)
                            if kb % 4 == 0:
                                emit_k(1, kb // 4)
                        sc = psA.tile([128, 2, 512], f32, tag="scores", name="sc")
                        ksl = slice(kb * 128, (kb + 1) * 128)
                        nc.tensor.matmul(
                            sc[:, 0, :],
                            K_sb[jt][sA:sA + 32, ksl],
                            Q_sb[jt][sA:sA + 32, qsl],
                            start=True, stop=True, tile_position=(sA, 0),
                        )
                        nc.tensor.matmul(
                            sc[:, 1, :],
                            K_sb[jt][sB:sB + 32, ksl],
                            Q_sb[jt][sB:sB + 32, qsl],
                            start=True, stop=True, tile_position=(sB, 0),
                        )
                        ex = exps.tile([128, 2, 512], f32r, tag="ex", name="ex")
                        nc.scalar.activation(out=ex, in_=sc, func=Act.Exp, scale=SCALE)
                        if pending is not None:
                            pending()
                        pending = make_step(info, kb, ex)
                    wave_i += 1
                # drain the lag before this chunk's projection
                if pending is not None:
                    pending()
                    pending = None

                # ---- proj + bias + residual for this query chunk ----
                for j in range(2):
                    ps = psC.tile([128, 512], f32, tag="work", name="pps")
                    for kk in range(2):
                        nc.tensor.matmul(
                            ps,
                            wp_sb[kk][:, j * 128:(j + 1) * 128],
                            O_sb[kk][:, qsl],
                            start=(kk == 0), stop=(kk == 1),
                        )
                    nc.vector.tensor_scalar_add(
                        out=out_sb[j][:, qsl], in0=ps, scalar1=bp_sb[j],
                    )
                    nc.vector.tensor_add(
                        out=out_sb[j][:, qsl], in0=out_sb[j][:, qsl],
                        in1=xqt[j][:, qsl],
                    )
                    nc.sync.dma_start(
                        out=out_d[j * 128:(j + 1) * 128, qsl],
                        in_=out_sb[j][:, qsl],
                    )

    nc.compile()
    return nc


_NC_CACHE = None


def kernel(x, gamma, beta, w_qkv, b_qkv, w_proj, b_proj):
    global LAST_RESULTS, _NC_CACHE
    from concourse.bass_utils import run_bass_kernel_spmd

    x = np.ascontiguousarray(np.asarray(x, np.float32))
    gamma = np.asarray(gamma, np.float32)
    beta = np.asarray(beta, np.float32)
    w_qkv = np.asarray(w_qkv, np.float32)
    b_qkv = np.asarray(b_qkv, np.float32)
    w_proj = np.asarray(w_proj, np.float32)
    b_proj = np.asarray(b_proj, np.float32)

    # Fold GroupNorm's gamma/beta into the QKV conv (per-voxel linear):
    #   qkv(hn*g + b) = (w*g) @ hn + (b_qkv + w @ b)
    w_f = w_qkv * gamma[None, :]
    b_f = b_qkv + w_qkv @ beta
    wqT = np.ascontiguousarray(w_f[0:C].T)
    wkT = np.ascontiguousarray(w_f[C:2 * C].T)
    wvT = np.ascontiguousarray(w_f[2 * C:3 * C].T)
    wpT = np.ascontiguousarray(w_proj.T)
    bq = np.ascontiguousarray(b_f[0:C].reshape(C, 1))
    bk = np.ascontiguousarray(b_f[C:2 * C].reshape(C, 1))
    bv = np.ascontiguousarray(b_f[2 * C:3 * C].reshape(1, C))
    bp = np.ascontiguousarray(b_proj.reshape(C, 1))

    part = np.arange(128)
    gmap = np.zeros((2, 128, GROUPS), np.float32)
    bmap = np.zeros((2, GROUPS, 128), np.float32)
    for j in range(2):
        g_of_p = (part + 128 * j) // (C // GROUPS)
        gmap[j, part, g_of_p] = 1.0
        bmap[j, g_of_p, part] = 1.0

    xf = x.reshape(B, C, N)
    in_maps = []
    for core in range(N_CORES):
        b, qs = core // 4, core % 4
        in_maps.append({
            "x": np.ascontiguousarray(xf[b]),
            "xq": np.ascontiguousarray(xf[b][:, qs * NQ:(qs + 1) * NQ]),
            "wqT": wqT, "wkT": wkT, "wvT": wvT, "wpT": wpT,
            "bq": bq, "bk": bk, "bv": bv, "bp": bp,
            "gmap": gmap, "bmap": bmap,
        })

    if _NC_CACHE is None:
        _NC_CACHE = _build_program()
    res = run_bass_kernel_spmd(_NC_CACHE, in_maps, list(range(N_CORES)))
    LAST_RESULTS = res

    out = np.empty((B, C, N), np.float32)
    for core in range(N_CORES):
        b, qs = core // 4, core % 4
        out[b][:, qs * NQ:(qs + 1) * NQ] = res.results[core]["out"]
    return out.reshape(B, C, 16, 16, 16)


# revision 58
# speedup vs baseline: 1.0002x; 1.0002x over previous
"""AttentionBlock3D (GroupNorm + 8-head attention + proj + residual) on 8 trn2 cores.

Sharding: core i handles (batch b = i//4, query-quarter qs = i%4).
Each core redundantly computes GroupNorm + full K/V for its batch (cheap:
~13% extra FLOPs), and exclusively computes Q/attention/projection for its
1024 spatial positions. No inter-core communication; the host concatenates
the 8 output slices.

Per-core design (ACT-bound: 33.5M softmax exps at 1 elem/lane/cycle):
  - All matmuls in fp32r (full PE rate at moving-dim >= 256, ~19-bit acc).
  - hn [C,N], K [C,N], Q [C,NQ] channel-major: head h's k^T/q^T are rows
    32h..32h+32, directly usable as QK^T operands -- no transposes anywhere.
  - QK^T pairs two heads via tile_position row groups (32-wide K strips).
  - V^T [N, 8, 33]: 32 value columns + a ones column per head, so one AV
    matmul accumulates both O^T and the softmax denominator (row 32).
  - Softmax skips the max-subtraction (scores are O(1) by construction).
  - exp(scale*s) on ACT reads scores straight from PSUM [128,1024] and
    writes fp32r SBUF tiles that feed AV.
  - AV matmuls lag the QK/exp stream by one step so wave boundaries never
    stall the exp pipeline on the in-order PE queue.
  - Denominator reciprocal is broadcast across dh rows by the idle GPSIMD
    engine (physical-partition-0 broadcast).
  - GroupNorm: bn_stats + per-group combine via a tiny one-hot matmul;
    gamma/beta and 1/sqrt(dh) are folded into weights/exp-scale on the host.
"""

import numpy as np

B, C, N = 2, 256, 4096
HEADS, GROUPS = 8, 8
DH = C // HEADS  # 32
NQ = N // 4      # queries per core
EPS = 1e-5
N_CORES = 8
NKB = N // 128   # 32 key blocks
SCALE = 1.0 / float(np.sqrt(DH))

LAST_RESULTS = None  # BassKernelResults of the most recent run (for test.py)


def _build_program():
    import concourse.bass as bass
    import concourse.bacc as bacc
    import concourse.tile as tile
    from concourse import mybir

    f32 = mybir.dt.float32
    f32r = mybir.dt.float32r
    Alu = mybir.AluOpType
    Act = mybir.ActivationFunctionType

    nc = bass.Bass("TRN2", target_bir_lowering=False, use_seq_codegen=True)

    # ---- DRAM I/O ----
    x_d = nc.dram_tensor("x", [C, N], f32, kind="ExternalInput")
    xq_d = nc.dram_tensor("xq", [C, NQ], f32, kind="ExternalInput")
    wqT_d = nc.dram_tensor("wqT", [C, C], f32, kind="ExternalInput")
    wkT_d = nc.dram_tensor("wkT", [C, C], f32, kind="ExternalInput")
    wvT_d = nc.dram_tensor("wvT", [C, C], f32, kind="ExternalInput")
    wpT_d = nc.dram_tensor("wpT", [C, C], f32, kind="ExternalInput")
    bq_d = nc.dram_tensor("bq", [C, 1], f32, kind="ExternalInput")
    bk_d = nc.dram_tensor("bk", [C, 1], f32, kind="ExternalInput")
    bv_d = nc.dram_tensor("bv", [1, C], f32, kind="ExternalInput")
    bp_d = nc.dram_tensor("bp", [C, 1], f32, kind="ExternalInput")
    gmap_d = nc.dram_tensor("gmap", [2, 128, GROUPS], f32, kind="ExternalInput")
    bmap_d = nc.dram_tensor("bmap", [2, GROUPS, 128], f32, kind="ExternalInput")
    out_d = nc.dram_tensor("out", [C, NQ], f32, kind="ExternalOutput")

    with tile.TileContext(nc) as tc:
        with (
            tc.tile_pool(name="const", bufs=1) as const,
            tc.tile_pool(name="data", bufs=1) as data,
            tc.tile_pool(name="tmp", bufs=2) as tmp,
            tc.tile_pool(name="exps", bufs=3) as exps,
            tc.tile_pool(name="psA", bufs=2, space="PSUM") as psA,
            tc.tile_pool(name="psB", bufs=2, space="PSUM") as psB,
            tc.tile_pool(name="psC", bufs=2, space="PSUM") as psC,
        ):
            # ---- small constants (needed by the GN stats path) ----
            bq_sb = [const.tile([128, 1], f32, name=f"bq{j}") for j in range(2)]
            bk_sb = [const.tile([128, 1], f32, name=f"bk{j}") for j in range(2)]
            bp_sb = [const.tile([128, 1], f32, name=f"bp{j}") for j in range(2)]
            for j in range(2):
                nc.gpsimd.dma_start(out=bq_sb[j], in_=bq_d[j * 128:(j + 1) * 128, :])
                nc.gpsimd.dma_start(out=bk_sb[j], in_=bk_d[j * 128:(j + 1) * 128, :])
                nc.gpsimd.dma_start(out=bp_sb[j], in_=bp_d[j * 128:(j + 1) * 128, :])
            bv_sb = const.tile([128, C], f32)
            nc.gpsimd.dma_start(out=bv_sb, in_=bv_d[:, :].to_broadcast([128, C]))
            gmap_sb = [const.tile([128, GROUPS], f32, name=f"gmap{j}") for j in range(2)]
            bmap_sb = [const.tile([GROUPS, 128], f32, name=f"bmap{j}") for j in range(2)]
            for j in range(2):
                gstg = tmp.tile([128, GROUPS], f32, tag="gstg", name="gstg", bufs=2)
                nc.gpsimd.dma_start(out=gstg, in_=gmap_d[j])
                nc.vector.tensor_copy(out=gmap_sb[j], in_=gstg)
                bstg = tmp.tile([GROUPS, 128], f32, tag="bstg", name="bstg", bufs=2)
                nc.gpsimd.dma_start(out=bstg, in_=bmap_d[j])
                nc.vector.tensor_copy(out=bmap_sb[j], in_=bstg)

            # ACT table prewarm: a dummy exp at t=0 pulls the ln/exp table
            # load off the critical path (GN rstd and softmax share the set)
            warm = tmp.tile([8, 1], f32, tag="warm", bufs=1)
            nc.vector.memset(warm, 0.0)
            nc.scalar.activation(out=warm, in_=warm, func=Act.Exp)

            # ---- load x (chunked so stats start before the full load) ----
            xt = [data.tile([128, N], f32, name=f"xt{j}") for j in range(2)]
            xqt = [data.tile([128, NQ], f32, name=f"xqt{j}") for j in range(2)]
            for j in range(2):
                eng = nc.sync if j == 0 else nc.scalar
                for ch in range(4):
                    csl = slice(ch * 1024, (ch + 1) * 1024)
                    eng.dma_start(out=xt[j][:, csl], in_=x_d[j * 128:(j + 1) * 128, csl])
                eng.dma_start(out=xqt[j], in_=xq_d[j * 128:(j + 1) * 128, :])

            # normalized activations (fp32r, produced by the affine pass below)
            hn = [data.tile([128, N], f32r, name=f"hn{j}") for j in range(2)]
            hnq = [data.tile([128, NQ], f32r, name=f"hnq{j}") for j in range(2)]

            # ---- GroupNorm statistics via bn_stats (one DVE pass over x) ----
            # st[j] = per-partition [mean, E[x^2]]
            st = [tmp.tile([128, 2], f32, name=f"st{j}", tag="st", bufs=2) for j in range(2)]
            for j in range(2):
                bnst = tmp.tile([128, 8, 6], f32, tag="bnst", bufs=2, name="bnst")
                for sub in range(8):
                    nc.vector.bn_stats(
                        out=bnst[:, sub, :],
                        in_=xt[j][:, sub * 512:(sub + 1) * 512],
                    )
                mv = tmp.tile([128, 2], f32, tag="mv", bufs=2, name="mv")
                nc.vector.bn_aggr(out=mv, in_=bnst)
                nc.vector.tensor_copy(out=st[j][:, 0:1], in_=mv[:, 0:1])
                nc.vector.tensor_mul(out=st[j][:, 1:2], in0=mv[:, 0:1], in1=mv[:, 0:1])
                nc.vector.tensor_add(out=st[j][:, 1:2], in0=st[j][:, 1:2], in1=mv[:, 1:2])
            stats_ps = psC.tile([GROUPS, 2], f32, tag="work")
            for j in range(2):
                nc.tensor.matmul(
                    stats_ps, gmap_sb[j], st[j], start=(j == 0), stop=(j == 1),
                )
            # per-group mean / rstd (tiny DVE/ACT ops on 8 partitions)
            gs = tmp.tile([GROUPS, 2], f32, tag="gs", bufs=1)
            nc.vector.tensor_copy(out=gs, in_=stats_ps)
            inv_n = 1.0 / (C // GROUPS)  # each group sums 32 per-partition means
            ms = tmp.tile([GROUPS, 2], f32, tag="ms", bufs=1)  # [mu | rstd]
            nc.vector.tensor_scalar_mul(out=ms[:, 0:1], in0=gs[:, 0:1], scalar1=inv_n)
            ve = tmp.tile([GROUPS, 1], f32, tag="ve", bufs=1)
            nc.vector.tensor_scalar_mul(out=ve, in0=gs[:, 1:2], scalar1=inv_n)
            musq = tmp.tile([GROUPS, 1], f32, tag="musq", bufs=1)
            nc.vector.tensor_mul(out=musq, in0=ms[:, 0:1], in1=ms[:, 0:1])
            nc.vector.tensor_sub(out=ve, in0=ve, in1=musq)
            nc.vector.tensor_scalar_add(out=ve, in0=ve, scalar1=EPS)
            # rstd = exp(-0.5*ln(v)): Ln/Exp share one activation table set
            # with the softmax exps, so no extra table load on the critical path
            sd = tmp.tile([GROUPS, 1], f32, tag="sd", bufs=1)
            nc.scalar.activation(out=sd, in_=ve, func=Act.Ln)
            r0 = tmp.tile([GROUPS, 1], f32, tag="r0", bufs=1)
            nc.scalar.activation(out=r0, in_=sd, func=Act.Exp, scale=-0.5)
            # one Newton polish: r = r0 * (1.5 - 0.5 * ve * r0^2)
            t_nw = tmp.tile([GROUPS, 1], f32, tag="t_nw", bufs=1)
            nc.vector.tensor_mul(out=t_nw, in0=r0, in1=r0)
            nc.vector.tensor_mul(out=t_nw, in0=t_nw, in1=ve)
            nc.vector.tensor_scalar(
                out=t_nw, in0=t_nw, scalar1=-0.5, scalar2=1.5,
                op0=Alu.mult, op1=Alu.add,
            )
            nc.vector.tensor_mul(out=ms[:, 1:2], in0=r0, in1=t_nw)

            # ---- weights: f32 staging (SP queue), DVE copies deferred ----
            # (walrus requires fp32r matmul operands to be *produced* as fp32r)
            wq_sb = [const.tile([128, C], f32r, name=f"wq{j}") for j in range(2)]
            wk_sb = [const.tile([128, C], f32r, name=f"wk{j}") for j in range(2)]
            wv_sb = [const.tile([128, C], f32r, name=f"wv{j}") for j in range(2)]
            wp_sb = [const.tile([128, C], f32r, name=f"wp{j}") for j in range(2)]
            wstgs = {}
            for j in range(2):
                for wi, (wd, wt) in enumerate(((wqT_d, wq_sb), (wkT_d, wk_sb),
                                               (wvT_d, wv_sb), (wpT_d, wp_sb))):
                    wstg = tmp.tile([128, C], f32, tag="wstg", name="wstg", bufs=8)
                    nc.sync.dma_start(out=wstg, in_=wd[j * 128:(j + 1) * 128, :])
                    wstgs[(wi, j)] = (wstg, wt)

            def copy_w(wi, j):
                wstg, wt = wstgs[(wi, j)]
                nc.vector.tensor_copy(out=wt[j], in_=wstg)

            # broadcast (mu, rstd) to per-partition columns
            musc = []
            for j in range(2):
                bc_ps = psC.tile([128, 2], f32, tag="work", name=f"bc_ps{j}")
                nc.tensor.matmul(bc_ps, bmap_sb[j], ms, start=True, stop=True)
                m = tmp.tile([128, 3], f32, tag="musc", bufs=2, name=f"musc{j}")
                nc.vector.tensor_copy(out=m[:, 0:2], in_=bc_ps)
                nc.vector.tensor_mul(out=m[:, 2:3], in0=m[:, 0:1], in1=m[:, 1:2])
                nc.vector.tensor_scalar_mul(out=m[:, 2:3], in0=m[:, 2:3], scalar1=-1.0)
                musc.append(m)

            # ---- normalize (gamma/beta pre-folded into weights on host) ----
            # xt/xqt stay raw fp32 (xqt doubles as the residual source)
            for j in range(2):
                copy_w(0, j)  # wq
                copy_w(1, j)  # wk
            # hnq normalize on ACT (idle, runs parallel to the DVE hn chain);
            # then fold the proj bias into the residual source in place, so the
            # projection epilogue is a single tensor_add
            for j in range(2):
                nc.scalar.activation(
                    out=hnq[j], in_=xqt[j], func=Act.Identity,
                    bias=musc[j][:, 2:3], scale=musc[j][:, 1:2],
                )


            def norm_half(hh):
                hsl = slice(hh * (N // 2), (hh + 1) * (N // 2))
                for j in range(2):
                    nc.vector.tensor_scalar(
                        out=hn[j][:, hsl], in0=xt[j][:, hsl],
                        scalar1=musc[j][:, 0:1], scalar2=musc[j][:, 1:2],
                        op0=Alu.subtract, op1=Alu.mult,
                    )

            K_sb = [data.tile([128, N], f32r, name=f"K{j}") for j in range(2)]
            Q_sb = [data.tile([128, NQ], f32r, name=f"Q{j}") for j in range(2)]

            def emit_q(j, n):
                ps = psC.tile([128, 512], f32, tag="work", name="qps")
                for kk in range(2):
                    nc.tensor.matmul(
                        ps,
                        wq_sb[kk][:, j * 128:(j + 1) * 128],
                        hnq[kk][:, n * 512:(n + 1) * 512],
                        start=(kk == 0), stop=(kk == 1),
                    )
                nc.vector.tensor_scalar_add(
                    out=Q_sb[j][:, n * 512:(n + 1) * 512], in0=ps,
                    scalar1=bq_sb[j],
                )

            def emit_k(j, n):
                ps = psC.tile([128, 512], f32, tag="work", name="kps")
                for kk in range(2):
                    nc.tensor.matmul(
                        ps,
                        wk_sb[kk][:, j * 128:(j + 1) * 128],
                        hn[kk][:, n * 512:(n + 1) * 512],
                        start=(kk == 0), stop=(kk == 1),
                    )
                nc.vector.tensor_scalar_add(
                    out=K_sb[j][:, n * 512:(n + 1) * 512], in0=ps,
                    scalar1=bk_sb[j],
                )

            for n in range(NQ // 512):
                emit_q(0, n)
            norm_half(0)
            for n in range(4):
                emit_k(0, n)
            norm_half(1)
            for n in range(4, N // 512):
                emit_k(0, n)
            for j in range(2):
                copy_w(2, j)  # wv
                copy_w(3, j)  # wp

            # ---- V^T = (hn)^T @ Wv^T + bv, layout [128, kb, 8, 33] ----
            # per head: 32 value columns + a ones column, so a single AV matmul
            # ([128, 33] lhsT) also produces the softmax denominator in row 32.
            # The per-kb V matmuls are emitted inside the first attention wave
            # so the exp stream starts as early as possible.
            V_sb = data.tile([128, NKB, HEADS, DH + 1], f32r)
            vones = const.tile([128, NKB * HEADS], f32)
            nc.vector.memset(vones, 1.0)
            nc.vector.tensor_copy(
                out=V_sb[:, :, :, DH:DH + 1],
                in_=vones.rearrange("p (kb h o) -> p kb h o", h=HEADS, o=1),
            )

            def emit_v(kb):
                ps = psC.tile([128, C], f32, tag="work", name="vps")
                for kk in range(2):
                    nc.tensor.matmul(
                        ps,
                        hn[kk][:, kb * 128:(kb + 1) * 128],
                        wv_sb[kk],
                        start=(kk == 0), stop=(kk == 1),
                    )
                nc.vector.tensor_add(
                    out=V_sb[:, kb, :, 0:DH],
                    in0=ps.rearrange("p (h x) -> p h x", h=HEADS),
                    in1=bv_sb.rearrange("p (h x) -> p h x", h=HEADS),
                )

            # ---- attention: chunk-outer so proj/output can drain per chunk.
            # AV matmuls lag the QK/exp stream by one step so the next wave's
            # QKs slot in front of the previous wave's last AV on the in-order
            # PE queue (kills the wave-boundary exp stall). The normalize tail
            # rides the lagged last AV.
            O_sb = [data.tile([128, NQ], f32r, name=f"O{j}") for j in range(2)]
            out_sb = [data.tile([128, NQ], f32, name=f"outsb{j}") for j in range(2)]

            def make_step(info, kb, ex):
                def emit():
                    if info["oda"] is None:
                        info["oda"] = psB.tile([DH + 1, 512], f32, tag="otd", name="oda")
                        info["odb"] = psB.tile([DH + 1, 512], f32, tag="otd", name="odb")
                    first, last = (kb == 0), (kb == NKB - 1)
                    nc.tensor.matmul(
                        info["oda"], V_sb[:, kb, info["hA"], :], ex[:, 0, :],
                        start=first, stop=last, skip_group_check=True,
                        tile_position=(0, 0),
                    )
                    nc.tensor.matmul(
                        info["odb"], V_sb[:, kb, info["hB"], :], ex[:, 1, :],
                        start=first, stop=last, skip_group_check=True,
                        tile_position=(0, 0),
                    )
                    if last:
                        # normalize: O^T * (1/denom); both reciprocals land on
                        # partition 0 (free-dim separated) because gpsimd
                        # partition_broadcast replicates physical partition 0
                        oda, odb = info["oda"], info["odb"]
                        rc = tmp.tile([32, 2, 512], f32, tag="rc", name="rc", bufs=1)
                        nc.vector.reciprocal(out=rc[0:1, 0, :], in_=oda[DH:DH + 1, :])
                        nc.vector.reciprocal(out=rc[0:1, 1, :], in_=odb[DH:DH + 1, :])
                        nb = tmp.tile([32, 2, 512], f32, tag="nb", name="nb", bufs=1)
                        nc.gpsimd.partition_broadcast(nb[:, 0, :], rc[0:1, 0, :])
                        nc.gpsimd.partition_broadcast(nb[:, 1, :], rc[0:1, 1, :])
                        nc.vector.tensor_mul(
                            out=O_sb[info["jt"]][info["sA"]:info["sA"] + 32, info["qsl"]],
                            in0=oda[0:DH, :], in1=nb[:, 0, :],
                        )
                        nc.vector.tensor_mul(
                            out=O_sb[info["jt"]][info["sB"]:info["sB"] + 32, info["qsl"]],
                            in0=odb[0:DH, :], in1=nb[:, 1, :],
                        )
                return emit

            # fold the proj bias into the residual source (hnq already read
            # raw xqt; DVE has slack once the attention stream is running)
            for j in range(2):
                nc.vector.tensor_scalar_add(
                    out=xqt[j], in0=xqt[j], scalar1=bp_sb[j],
                )

            wave_i = 0
            pending = None
            for c in range(NQ // 512):
                qsl = slice(c * 512, (c + 1) * 512)
                for p in range(4):
                    hA, hB = 2 * p, 2 * p + 1
                    info = {
                        "hA": hA, "hB": hB, "jt": hA // 4,
                        "sA": 32 * (hA % 4), "sB": 32 * (hB % 4),
                        "qsl": qsl, "oda": None, "odb": None,
                    }
                    sA, sB, jt = info["sA"], info["sB"], info["jt"]
                    for kb in range(NKB):
                        if wave_i == 0:
                            emit_v(kb)
                        elif wave_i == 2:
                            # K/Q j=1 computed just-in-time for this wave's QKs
                            if kb < 2:
                                emit_q(1, kb)
                            if kb % 4 == 0:
                                emit_k(1, kb // 4)
                        sc = psA.tile([128, 2, 512], f32, tag="scores", name="sc")
                        ksl = slice(kb * 128, (kb + 1) * 128)
                        nc.tensor.matmul(
                            sc[:, 0, :],
                            K_sb[jt][sA:sA + 32, ksl],
                            Q_sb[jt][sA:sA + 32, qsl],
                            start=True, stop=True, tile_position=(sA, 0),
                        )
                        nc.tensor.matmul(
                            sc[:, 1, :],
                            K_sb[jt][sB:sB + 32, ksl],
                            Q_sb[jt][sB:sB + 32, qsl],
                            start=True, stop=True, tile_position=(sB, 0),
                        )
                        ex = exps.tile([128, 2, 512], f32r, tag="ex", name="ex")
                        nc.scalar.activation(out=ex, in_=sc, func=Act.Exp, scale=SCALE)
                        if pending is not None:
                            pending()
                        pending = make_step(info, kb, ex)
                    wave_i += 1
                # drain the lag before this chunk's projection
                if pending is not None:
                    pending()
                    pending = None

                # ---- proj + bias + residual for this query chunk ----
                for j in range(2):
                    ps = psC.tile([128, 512], f32, tag="work", name="pps")
                    for kk in range(2):
                        nc.tensor.matmul(
                            ps,
                            wp_sb[kk][:, j * 128:(j + 1) * 128],
                            O_sb[kk][:, qsl],
                            start=(kk == 0), stop=(kk == 1),
                        )
                    nc.vector.tensor_add(
                        out=out_sb[j][:, qsl], in0=ps, in1=xqt[j][:, qsl],
                    )
                    nc.sync.dma_start(
                        out=out_d[j * 128:(j + 1) * 128, qsl],
                        in_=out_sb[j][:, qsl],
                    )

    nc.compile()
    return nc


_NC_CACHE = None


def kernel(x, gamma, beta, w_qkv, b_qkv, w_proj, b_proj):
    global LAST_RESULTS, _NC_CACHE
    from concourse.bass_utils import run_bass_kernel_spmd

    x = np.ascontiguousarray(np.asarray(x, np.float32))
    gamma = np.asarray(gamma, np.float32)
    beta = np.asarray(beta, np.float32)
    w_qkv = np.asarray(w_qkv, np.float32)
    b_qkv = np.asarray(b_qkv, np.float32)
    w_proj = np.asarray(w_proj, np.float32)
    b_proj = np.asarray(b_proj, np.float32)

    # Fold GroupNorm's gamma/beta into the QKV conv (per-voxel linear):
    #   qkv(hn*g + b) = (w*g) @ hn + (b_qkv + w @ b)
    w_f = w_qkv * gamma[None, :]
    b_f = b_qkv + w_qkv @ beta
    wqT = np.ascontiguousarray(w_f[0:C].T)
    wkT = np.ascontiguousarray(w_f[C:2 * C].T)
    wvT = np.ascontiguousarray(w_f[2 * C:3 * C].T)
    wpT = np.ascontiguousarray(w_proj.T)
    bq = np.ascontiguousarray(b_f[0:C].reshape(C, 1))
    bk = np.ascontiguousarray(b_f[C:2 * C].reshape(C, 1))
    bv = np.ascontiguousarray(b_f[2 * C:3 * C].reshape(1, C))
    bp = np.ascontiguousarray(b_proj.reshape(C, 1))

    part = np.arange(128)
    gmap = np.zeros((2, 128, GROUPS), np.float32)
    bmap = np.zeros((2, GROUPS, 128), np.float32)
    for j in range(2):
        g_of_p = (part + 128 * j) // (C // GROUPS)
        gmap[j, part, g_of_p] = 1.0
        bmap[j, g_of_p, part] = 1.0

    xf = x.reshape(B, C, N)
    in_maps = []
    for core in range(N_CORES):
        b, qs = core // 4, core % 4
        in_maps.append({
            "x": np.ascontiguousarray(xf[b]),
            "xq": np.ascontiguousarray(xf[b][:, qs * NQ:(qs + 1) * NQ]),
            "wqT": wqT, "wkT": wkT, "wvT": wvT, "wpT": wpT,
            "bq": bq, "bk": bk, "bv": bv, "bp": bp,
            "gmap": gmap, "bmap": bmap,
        })

    if _NC_CACHE is None:
        _NC_CACHE = _build_program()
    res = run_bass_kernel_spmd(_NC_CACHE, in_maps, list(range(N_CORES)))
    LAST_RESULTS = res

    out = np.empty((B, C, N), np.float32)
    for core in range(N_CORES):
        b, qs = core // 4, core % 4
        out[b][:, qs * NQ:(qs + 1) * NQ] = res.results[core]["out"]
    return out.reshape(B, C, 16, 16, 16)
